# revision 50
# baseline (speedup 1.0000x reference)
"""KMoE feed-forward on 8 TRN2 NeuronCores.

Two fixed (input-independent) SPMD Bass programs — one per KMoE layer — with
host-side routing.  Layer-1 per-slot expert outputs (h_slots) stay on-device
between the two launches, and every data-dependent token->slot dispatch runs
on-device via gpsimd.dma_gather, so the only host<->device traffic is the
slot-packed x, per-group weight panels, router logits, small index tables and
the final output.  Device matmuls are bf16 with f32 PSUM accumulation.
"""
import numpy as np

D1 = D2 = 32
F1 = F2 = 64
E = 64
TOP_K = 2
N_CORES = 8
G = 8                      # slots per expert group (one weight panel entry)
S = 2560                   # fixed padded slot count (2048 + 64*(G-1) = 2496 max)
NG = S // G
NTOK = 1024                # tokens per core
RCH = 256                  # router token chunk (layer 1)
GCH = 256                  # slot gather chunk (layer 2)

_bf16 = None


def _bf16dt():
    global _bf16
    if _bf16 is None:
        import ml_dtypes
        _bf16 = np.dtype(ml_dtypes.bfloat16)
    return _bf16


# ---------------------------------------------------------------- host glue

def _route(logits):
    idx = np.argpartition(-logits, TOP_K - 1, axis=1)[:, :TOP_K]
    vals = np.take_along_axis(logits, idx, axis=1)
    order = np.argsort(-vals, axis=1, kind="stable")
    idx = np.take_along_axis(idx, order, axis=1)
    vals = np.take_along_axis(vals, order, axis=1)
    ex = np.exp(vals - vals.max(axis=1, keepdims=True))
    probs = ex / ex.sum(axis=1, keepdims=True)
    return idx.astype(np.int64), probs.astype(np.float32)


def _dispatch(idx, probs):
    """Expert-sorted slot assignment for one core."""
    n = idx.shape[0]
    ex = idx.reshape(-1)
    order = np.argsort(ex, kind="stable")
    counts = np.bincount(ex, minlength=E)
    padded = ((counts + G - 1) // G) * G
    total = int(padded.sum())
    assert total <= S, (total, S)
    off_pad = np.zeros(E, dtype=np.int64)
    off_pad[1:] = np.cumsum(padded)[:-1]
    off_raw = np.zeros(E, dtype=np.int64)
    off_raw[1:] = np.cumsum(counts)[:-1]
    ex_sorted = ex[order]
    pos_in_e = np.arange(2 * n) - off_raw[ex_sorted]
    slot_sorted = off_pad[ex_sorted] + pos_in_e
    tok_of_slot = np.zeros(S, dtype=np.int64)
    gate_of_slot = np.zeros(S, dtype=np.float32)
    tok_of_slot[slot_sorted] = order // 2
    gate_of_slot[slot_sorted] = probs.reshape(-1)[order]
    slot_of_pair = np.zeros((n, 2), dtype=np.int64)
    slot_of_pair[order // 2, order % 2] = slot_sorted
    expert_of_group = np.zeros(NG, dtype=np.int64)
    reps = (padded // G).astype(np.int64)
    eog = np.repeat(np.arange(E), reps)
    expert_of_group[: eog.shape[0]] = eog
    return tok_of_slot, gate_of_slot, expert_of_group, slot_of_pair


def _wrap16(v):
    """[n] int -> [128, n//16] int16 dma_gather index layout."""
    v = np.asarray(v, dtype=np.int16)
    lay = v.reshape(-1, 16).T            # [16, n//16]
    return np.ascontiguousarray(np.tile(lay, (8, 1)))


def _exact_logits(X, idx1c, probs1c, A_up, B_up, s_up, bias_up, W_down):
    """Exact f32 layer-2 router logits for a subset of tokens.

    X [M,32,32]; idx1c/probs1c [M,2] layer-1 routing of those tokens."""
    from scipy.special import erf
    Ag = A_up[idx1c]                     # [M,2,64,32]
    Bg = B_up[idx1c]
    U = np.einsum('mkoi,mij->mkoj', Ag, X, optimize=True)
    Y = np.einsum('mkoj,mkpj->mkop', U, Bg, optimize=True)
    h = ((Y * probs1c[:, :, None, None]).sum(1).reshape(X.shape[0], 4096)
         * s_up + bias_up[None, :])
    hg = 0.5 * h * (1.0 + erf(h / np.sqrt(2.0)))
    return (hg @ W_down.T).astype(np.float32)


_QQ = np.arange(128)
_BB32 = np.arange(32)
# h_slots row swizzle: stored flat[128*b + q] = h[(o1, p1)] with
# o1 = b + 32*(q//64), p1 = q%64  (i.e. flat(o1,p1) = 128*(o1%32)+64*(o1//32)+p1)
_FR = ((_BB32[None, :] + 32 * (_QQ[:, None] // 64)) * 64 + _QQ[:, None] % 64)  # [128,32]
_SIGMA = 32 * (np.arange(64) % 2) + np.arange(64) // 2  # wa1 panel column perm


# ------------------------------------------------------------- bass programs

def _build_l1(nc):
    import concourse.mybir as mybir
    import concourse.tile as tile

    bf = mybir.dt.bfloat16
    f32 = mybir.dt.float32
    i16 = mybir.dt.int16

    xs_d = nc.dram_tensor("xs", (32, S * 32), bf, kind="ExternalInput")
    wbp_d = nc.dram_tensor("wbp", (32, NG * 64), bf, kind="ExternalInput")
    wap_d = nc.dram_tensor("wap", (32, NG * 64), bf, kind="ExternalInput")
    wdp_d = nc.dram_tensor("wdp", (128, 32 * 64), bf, kind="ExternalInput")
    bias_d = nc.dram_tensor("biasT", (128, 32), bf, kind="ExternalInput")
    pa_d = nc.dram_tensor("pa", (128, NTOK // 16), i16, kind="ExternalInput")
    pb_d = nc.dram_tensor("pb", (128, NTOK // 16), i16, kind="ExternalInput")
    hso_d = nc.dram_tensor("hso", (NG, G, 64, 64), bf, kind="ExternalOutput")
    lg_d = nc.dram_tensor("lg", (64, NTOK), f32, kind="ExternalOutput")

    with tile.TileContext(nc) as tc:
        with tc.tile_pool(name="const", bufs=1) as cp, \
             tc.tile_pool(name="dram", bufs=1, space="DRAM") as dp, \
             tc.tile_pool(name="xstr", bufs=2) as xsp, \
             tc.tile_pool(name="vs", bufs=3) as vp, \
             tc.tile_pool(name="hp", bufs=3) as hp, \
             tc.tile_pool(name="ga", bufs=2) as gap, \
             tc.tile_pool(name="gb", bufs=2) as gbp, \
             tc.tile_pool(name="lo", bufs=2) as lop, \
             tc.tile_pool(name="sg", bufs=1) as sgp, \
             tc.tile_pool(name="ps1", bufs=2, space="PSUM") as p1, \
             tc.tile_pool(name="ps2", bufs=2, space="PSUM") as p2, \
             tc.tile_pool(name="psr", bufs=2, space="PSUM") as pr:
            hs = dp.tile([NG, G, 64, 64], bf)          # h_slots (internal DRAM)
            wbp = cp.tile([32, NG * 64], bf)
            nc.sync.dma_start(wbp[:], wbp_d[:])
            wap = cp.tile([32, NG * 64], bf)
            nc.sync.dma_start(wap[:], wap_d[:])
            wdp = cp.tile([128, 32 * 64], bf)
            nc.sync.dma_start(wdp[:], wdp_d[:])
            biasT = cp.tile([128, 32], bf)
            nc.sync.dma_start(biasT[:], bias_d[:])
            pa = cp.tile([128, NTOK // 16], i16)
            nc.sync.dma_start(pa[:], pa_d[:])
            pb = cp.tile([128, NTOK // 16], i16)
            nc.sync.dma_start(pb[:], pb_d[:])

            CHG = 16                       # groups per streamed xs chunk
            for g in range(NG):
                if g % CHG == 0:
                    xs = xsp.tile([32, CHG * G * 32], bf, tag="xsch")
                    nc.sync.dma_start(
                        xs[:], xs_d[:, g * G * 32:(g + CHG) * G * 32])
                ps1 = p1.tile([32, 512], mybir.dt.float32)
                for k in range(G):
                    s = (g % CHG) * G + k
                    nc.tensor.matmul(ps1[:, k * 64:(k + 1) * 64],
                                     xs[:, s * 32:(s + 1) * 32],
                                     wbp[:, g * 64:(g + 1) * 64],
                                     start=True, stop=True)
                vsb = vp.tile([32, 512], bf, tag="vsb")
                nc.vector.tensor_copy(vsb[:], ps1[:])
                ps2 = p2.tile([64, 512], mybir.dt.float32)
                nc.tensor.matmul(ps2[:], wap[:, g * 64:(g + 1) * 64], vsb[:],
                                 start=True, stop=True)
                hsb = hp.tile([64, 512], bf, tag="hsb")
                nc.vector.tensor_copy(hsb[:], ps2[:])
                # h_slots row flat = 64*p + j  (p = psum partition, sigma-mapped)
                nc.sync.dma_start(hs[g].rearrange("k p j -> p k j"), hsb[:])

            hs_rows = hs[:].rearrange("g k p j -> (g k) (p j)")
            for c in range(NTOK // RCH):
                ha = gap.tile([128, 32, RCH], bf, tag="ha")
                nc.gpsimd.dma_gather(ha[:], hs_rows,
                                     pa[:, c * (RCH // 16):(c + 1) * (RCH // 16)],
                                     RCH, RCH, 4096, transpose=True)
                hb = gbp.tile([128, 32, RCH], bf, tag="hb")
                nc.gpsimd.dma_gather(hb[:], hs_rows,
                                     pb[:, c * (RCH // 16):(c + 1) * (RCH // 16)],
                                     RCH, RCH, 4096, transpose=True)
                nc.vector.tensor_add(ha[:], ha[:], hb[:])
                bias_b = biasT[:].unsqueeze(2).broadcast_to((128, 32, RCH))
                nc.vector.tensor_add(ha[:], ha[:], bias_b)
                # gelu(x) ~= x * sigmoid(1.702 x); exact to O(5e-5) at |x|<<1
                sg = sgp.tile([128, 32, RCH], bf, tag="sg")
                nc.scalar.activation(sg[:], ha[:],
                                     mybir.ActivationFunctionType.Sigmoid,
                                     scale=1.702)
                nc.vector.tensor_mul(ha[:], ha[:], sg[:])
                pl = pr.tile([64, RCH], mybir.dt.float32)
                for b in range(32):
                    nc.tensor.matmul(pl[:], wdp[:, b * 64:(b + 1) * 64],
                                     ha[:, b, :], start=(b == 0), stop=(b == 31))
                lsb = lop.tile([64, RCH], mybir.dt.float32, tag="lsb")
                nc.vector.tensor_copy(lsb[:], pl[:])
                nc.sync.dma_start(lg_d[:, c * RCH:(c + 1) * RCH], lsb[:])

            # publish h_slots for launch 2 (device-resident output)
            nc.sync.dma_start(hso_d[:], hs[:])
    return nc


def _build_l2(nc):
    import concourse.mybir as mybir
    import concourse.tile as tile

    bf = mybir.dt.bfloat16
    i16 = mybir.dt.int16

    hs_d = nc.dram_tensor("hs", (NG, G, 64, 64), bf, kind="ExternalInput")
    wbp_d = nc.dram_tensor("wbp", (64, NG * 32), bf, kind="ExternalInput")
    wap_d = nc.dram_tensor("wap", (64, NG * 32), bf, kind="ExternalInput")
    bias_d = nc.dram_tensor("biasT", (128, 32), bf, kind="ExternalInput")
    ga_d = nc.dram_tensor("ga", (128, S // 16), i16, kind="ExternalInput")
    gb_d = nc.dram_tensor("gb", (128, S // 16), i16, kind="ExternalInput")
    g2_d = nc.dram_tensor("g2e", (128, S), bf, kind="ExternalInput")
    qa_d = nc.dram_tensor("qa", (128, NTOK // 16), i16, kind="ExternalInput")
    qb_d = nc.dram_tensor("qb", (128, NTOK // 16), i16, kind="ExternalInput")
    yt_d = nc.dram_tensor("yt", (128, 8, NTOK), bf, kind="ExternalOutput")

    with tile.TileContext(nc) as tc:
        with tc.tile_pool(name="const", bufs=1) as cp, \
             tc.tile_pool(name="dram", bufs=1, space="DRAM") as dp, \
             tc.tile_pool(name="xa", bufs=2) as xap, \
             tc.tile_pool(name="xb", bufs=2) as xbp, \
             tc.tile_pool(name="vs", bufs=3) as vp, \
             tc.tile_pool(name="ys", bufs=3) as yp, \
             tc.tile_pool(name="fin", bufs=1) as fp, \
             tc.tile_pool(name="sg", bufs=1) as sgp, \
             tc.tile_pool(name="ps1", bufs=2, space="PSUM") as p1, \
             tc.tile_pool(name="ps2", bufs=2, space="PSUM") as p2:
            ys = dp.tile([S // 16, 16, 32, 32], bf)    # y_slots (internal DRAM)
            wb2 = cp.tile([128, NG * 32], bf)
            nc.sync.dma_start(wb2[0:64, :], wbp_d[:])
            nc.sync.dma_start(wb2[64:128, :], wbp_d[:])
            wa2 = cp.tile([64, NG * 32], bf)
            nc.sync.dma_start(wa2[:], wap_d[:])
            biasT = cp.tile([128, 32], bf)
            nc.sync.dma_start(biasT[:], bias_d[:])
            g2e = cp.tile([128, S], bf)
            nc.sync.dma_start(g2e[:], g2_d[:])
            ga = cp.tile([128, S // 16], i16)
            nc.sync.dma_start(ga[:], ga_d[:])
            gb = cp.tile([128, S // 16], i16)
            nc.sync.dma_start(gb[:], gb_d[:])
            qa = cp.tile([128, NTOK // 16], i16)
            nc.sync.dma_start(qa[:], qa_d[:])
            qb = cp.tile([128, NTOK // 16], i16)
            nc.sync.dma_start(qb[:], qb_d[:])

            hs_rows = hs_d[:].rearrange("g k p j -> (g k) (p j)")
            for c in range(S // GCH):
                xa = xap.tile([128, 32, GCH], bf, tag="xa")
                nc.gpsimd.dma_gather(xa[:], hs_rows,
                                     ga[:, c * (GCH // 16):(c + 1) * (GCH // 16)],
                                     GCH, GCH, 4096, transpose=True)
                xb = xbp.tile([128, 32, GCH], bf, tag="xb")
                nc.gpsimd.dma_gather(xb[:], hs_rows,
                                     gb[:, c * (GCH // 16):(c + 1) * (GCH // 16)],
                                     GCH, GCH, 4096, transpose=True)
                nc.vector.tensor_add(xa[:], xa[:], xb[:])
                bias_b = biasT[:].unsqueeze(2).broadcast_to((128, 32, GCH))
                nc.vector.tensor_add(xa[:], xa[:], bias_b)
                sg = sgp.tile([128, 32, GCH], bf, tag="sg")
                nc.scalar.activation(sg[:], xa[:],
                                     mybir.ActivationFunctionType.Sigmoid,
                                     scale=1.702)
                nc.vector.tensor_mul(xa[:], xa[:], sg[:])
                gate_b = (g2e[:, c * GCH:(c + 1) * GCH]
                          .unsqueeze(1).broadcast_to((128, 32, GCH)))
                nc.vector.tensor_mul(xa[:], xa[:], gate_b)
                for pair in range(GCH // 16):
                    ps1 = p1.tile([64, 512], mybir.dt.float32)
                    for k in range(16):
                        sl = pair * 16 + k
                        gof = (c * GCH + sl) // G
                        nc.tensor.matmul(ps1[0:32, k * 32:(k + 1) * 32],
                                         xa[0:64, :, sl],
                                         wb2[0:64, gof * 32:(gof + 1) * 32],
                                         start=True, stop=True)
                        nc.tensor.matmul(ps1[32:64, k * 32:(k + 1) * 32],
                                         xa[64:128, :, sl],
                                         wb2[64:128, gof * 32:(gof + 1) * 32],
                                         start=True, stop=True)
                    vsb = vp.tile([64, 512], bf, tag="vsb")
                    nc.vector.tensor_copy(vsb[:], ps1[:])
                    ps2 = p2.tile([32, 512], mybir.dt.float32)
                    for t in range(2):
                        gof = (c * GCH) // G + pair * 2 + t
                        nc.tensor.matmul(ps2[:, t * 256:(t + 1) * 256],
                                         wa2[:, gof * 32:(gof + 1) * 32],
                                         vsb[:, t * 256:(t + 1) * 256],
                                         start=True, stop=True)
                    ysb = yp.tile([32, 512], bf, tag="ysb")
                    nc.vector.tensor_copy(ysb[:], ps2[:])
                    blk = (c * GCH) // 16 + pair
                    nc.sync.dma_start(ys[blk].rearrange("k o p -> o k p"), ysb[:])

            # non-transpose gather: ya[t%128, t//128, :] = y_slots[qa[t], :]
            # (transpose-mode faults for 2048B rows on this runtime)
            ys_rows = ys[:].rearrange("g k o p -> (g k) (o p)")
            ya = fp.tile([128, 8, NTOK], bf, tag="ya")
            nc.gpsimd.dma_gather(ya[:], ys_rows, qa[:], NTOK, NTOK, 1024)
            yb = fp.tile([128, 8, NTOK], bf, tag="yb")
            nc.gpsimd.dma_gather(yb[:], ys_rows, qb[:], NTOK, NTOK, 1024)
            nc.vector.tensor_add(ya[:], ya[:], yb[:])
            nc.sync.dma_start(yt_d[:], ya[:])
    return nc


# ------------------------------------------------------------------ runner

_MESH = None


def _mesh():
    global _MESH
    if _MESH is None:
        import jax
        from jax.sharding import Mesh
        _MESH = Mesh(np.asarray(jax.devices()[:N_CORES]), ("core",))
    return _MESH


class _Prog:
    """A finalized bass program with an AOT-compiled sharded executable."""

    def __init__(self, name, build_fn):
        import jax
        import concourse.bacc as bacc
        import concourse.mybir as mybir
        from concourse import bass2jax
        from jax.sharding import PartitionSpec, NamedSharding
        from jax.experimental.shard_map import shard_map

        bass2jax.install_neuronx_cc_hook()
        nc = bacc.Bacc(name=name, trn_type="TRN2")
        build_fn(nc)
        nc.finalize()
        self.nc = nc
        mesh = _mesh()
        P = PartitionSpec
        self.sh = NamedSharding(mesh, P("core"))

        partition_name = (nc.partition_id_tensor.name
                          if nc.partition_id_tensor else None)
        in_names, out_names, out_avals = [], [], []
        for alloc in nc.m.functions[0].allocations:
            if not isinstance(alloc, mybir.MemoryLocationSet):
                continue
            nm = alloc.memorylocations[0].name
            if alloc.kind == "ExternalInput":
                if nm != partition_name:
                    in_names.append(nm)
            elif alloc.kind == "ExternalOutput":
                out_names.append(nm)
                out_avals.append(jax.core.ShapedArray(
                    tuple(alloc.tensor_shape), mybir.dt.np(alloc.dtype)))
        self.in_names, self.out_names, self.out_avals = \
            in_names, out_names, out_avals
        n_params = len(in_names)
        all_in = in_names + out_names
        if partition_name is not None:
            all_in.append(partition_name)
        all_in_names = tuple(all_in)

        def _body(*args):
            operands = list(args)
            if partition_name is not None:
                operands.append(bass2jax.partition_id_tensor())
            outs = bass2jax._bass_exec_p.bind(
                *operands, out_avals=tuple(out_avals), in_names=all_in_names,
                out_names=tuple(out_names), lowering_input_output_aliases=(),
                sim_require_finite=True, sim_require_nnan=True, nc=nc)
            return tuple(outs)

        donate = tuple(range(n_params, n_params + len(out_names)))
        sharded = jax.jit(
            shard_map(_body, mesh=mesh,
                      in_specs=(P("core"),) * (n_params + len(out_names)),
                      out_specs=(P("core"),) * len(out_names), check_rep=False),
            donate_argnums=donate, keep_unused=True)
        avals = []
        for nm in in_names:
            a = self._gshape(nc, nm)
            avals.append(a)
        for a2 in out_avals:
            avals.append(jax.ShapeDtypeStruct(
                (N_CORES * a2.shape[0],) + tuple(a2.shape[1:]), a2.dtype,
                sharding=self.sh))
        self.compiled = sharded.lower(*avals).compile()

    @staticmethod
    def _gshape(nc, nm):
        import jax
        import concourse.mybir as mybir
        for alloc in nc.m.functions[0].allocations:
            if (isinstance(alloc, mybir.MemoryLocationSet)
                    and alloc.memorylocations[0].name == nm):
                shp = tuple(alloc.tensor_shape)
                return jax.ShapeDtypeStruct(
                    (N_CORES * shp[0],) + shp[1:], mybir.dt.np(alloc.dtype))
        raise KeyError(nm)

    def run(self, dev_ins):
        import jax
        import jax.numpy as jnp
        ins = [dev_ins[nm] for nm in self.in_names]
        zeros = [
            jax.jit(lambda s=tuple(a.shape), d=a.dtype:
                    jnp.zeros((N_CORES * s[0],) + s[1:], d),
                    out_shardings=self.sh)()
            for a in self.out_avals
        ]
        outs = self.compiled(*ins, *zeros)
        return dict(zip(self.out_names, outs))


_PROGS = None


def _progs():
    global _PROGS
    if _PROGS is None:
        _PROGS = (_Prog("kmoe_l1", _build_l1), _Prog("kmoe_l2", _build_l2))
    return _PROGS


try:
    _progs()          # import-time init: device discovery, program build, AOT
except Exception:     # fall back to lazy init inside kernel()
    _PROGS = None


# ------------------------------------------------------------------ kernel

def kernel(x, W_up, A_up, B_up, scale_up, bias_up,
           W_down, A_down, B_down, scale_down, bias_down):
    import sys
    import time
    import concourse.bacc as bacc

    _t = [time.time()]

    def _mark(tag):
        now = time.time()
        print(f"[kmoe] {tag}: {now - _t[0]:.2f}s", file=sys.stderr)
        _t[0] = now

    bf16 = _bf16dt()
    x = np.asarray(x, np.float32)
    orig_shape = x.shape
    x_flat = np.ascontiguousarray(x.reshape(-1, D1 * D2))
    W_up = np.asarray(W_up, np.float32)
    A_up = np.asarray(A_up, np.float32)
    B_up = np.asarray(B_up, np.float32)
    W_down = np.asarray(W_down, np.float32)
    A_down = np.asarray(A_down, np.float32)
    B_down = np.asarray(B_down, np.float32)
    s_up = float(np.asarray(scale_up).reshape(-1)[0])
    s_dn = float(np.asarray(scale_down).reshape(-1)[0])
    bias_up = np.asarray(bias_up, np.float32).reshape(-1)
    bias_dn = np.asarray(bias_down, np.float32).reshape(-1)

    # ---- layer-1 routing + dispatch (host, f32 exact)
    logits1 = x_flat.reshape(-1, D1 * D2) @ W_up.T
    idx1, probs1 = _route(logits1)

    wb1_all = B_up.transpose(0, 2, 1)                         # [E, 32(j), 64(p)]
    wa1_all = (A_up[:, _SIGMA, :] * s_up).transpose(0, 2, 1)  # [E, 32(i), 64(o')]

    # router panel:  wdp[q, b*64+e] = W_down[e, _FR[q,b]]
    wdp = np.ascontiguousarray(
        W_down[:, _FR].transpose(1, 2, 0)).reshape(128, 32 * 64).astype(bf16)
    biasT = np.ascontiguousarray(bias_up[_FR]).astype(bf16)

    import jax
    p1, p2 = _progs()
    devs = list(_mesh().devices.flat)

    def _assemble(shards_by_name, sh):
        out = {}
        for k, shards in shards_by_name.items():
            s0 = shards[0].shape
            out[k] = jax.make_array_from_single_device_arrays(
                (N_CORES * s0[0],) + tuple(s0[1:]), sh, shards)
        return out

    # per-core prep in a thread pool with put-as-you-go: numpy prep for all
    # cores runs concurrently (GIL released in big array ops) and each core's
    # tunnel transfer starts the moment its arrays are ready
    from concurrent.futures import ThreadPoolExecutor

    sop1_all = [None] * N_CORES

    def _prep1(c):
        t0 = c * NTOK
        tok, gate, eog, sop = _dispatch(idx1[t0:t0 + NTOK], probs1[t0:t0 + NTOK])
        sop1_all[c] = sop
        xc = x_flat[t0:t0 + NTOK]
        xst = xc.reshape(NTOK, 32, 32)[tok] * gate[:, None, None]
        xs = np.ascontiguousarray(xst.transpose(2, 0, 1)).reshape(32, S * 32)
        percore = {
            "xs": xs.astype(bf16),
            "wbp": np.ascontiguousarray(
                wb1_all[eog].transpose(1, 0, 2)).reshape(32, NG * 64).astype(bf16),
            "wap": np.ascontiguousarray(
                wa1_all[eog].transpose(1, 0, 2)).reshape(32, NG * 64).astype(bf16),
            "wdp": wdp, "biasT": biasT,
            "pa": _wrap16(sop[:, 0]), "pb": _wrap16(sop[:, 1]),
        }
        return {k: jax.device_put(v, devs[c]) for k, v in percore.items()}

    with ThreadPoolExecutor(max_workers=N_CORES) as pool:
        percores = list(pool.map(_prep1, range(N_CORES)))
    shards1 = {k: [pc[k] for pc in percores] for k in percores[0]}

    _mark("l1 host prep+h2d dispatch")
    dev1 = _assemble(shards1, p1.sh)
    outs1 = p1.run(dev1)
    hs_dev = outs1["hso"]                    # stays on device
    logits2 = np.asarray(outs1["lg"])        # [8*64, 1024] f32
    _mark("l1 run+lg fetch")

    # ---- layer-2 routing (host) from device logits
    logits2 = logits2.reshape(N_CORES, 64, NTOK).transpose(0, 2, 1)

    wb2_all = B_down.transpose(0, 2, 1)      # [E, 64(j), 32(p)]
    wa2_all = A_down.transpose(0, 2, 1)      # [E, 64(i), 32(o)]

    def _prep2(c):
        lgc = np.ascontiguousarray(logits2[c])          # [NTOK, 64]
        # device logits carry bf16-level noise; near-tied top-2/3 tokens get
        # exact f32 logits recomputed on host so expert selection matches f32.
        top3 = np.argpartition(-lgc, 2, axis=1)[:, :3]
        v3 = np.sort(np.take_along_axis(lgc, top3, axis=1), axis=1)
        gap23 = v3[:, 1] - v3[:, 0]                     # 2nd minus 3rd largest
        tau = 0.02 * float(lgc.std())
        amb = np.nonzero(gap23 < tau)[0]
        if amb.size:
            t0c = c * NTOK
            Xa = x_flat[t0c:t0c + NTOK].reshape(NTOK, 32, 32)[amb]
            lgc[amb] = _exact_logits(
                Xa, idx1[t0c + amb], probs1[t0c + amb],
                A_up, B_up, s_up, bias_up, W_down)
        idx2, probs2 = _route(lgc)
        tok, gate, eog, sop = _dispatch(idx2, probs2)
        sop1 = sop1_all[c]
        percore = {
            "wbp": np.ascontiguousarray(
                wb2_all[eog].transpose(1, 0, 2)).reshape(64, NG * 32).astype(bf16),
            "wap": np.ascontiguousarray(
                wa2_all[eog].transpose(1, 0, 2)).reshape(64, NG * 32).astype(bf16),
            "biasT": biasT,
            "ga": _wrap16(sop1[tok, 0]),
            "gb": _wrap16(sop1[tok, 1]),
            "g2e": np.ascontiguousarray(
                np.broadcast_to(gate[None, :], (128, S))).astype(bf16),
            "qa": _wrap16(sop[:, 0]),
            "qb": _wrap16(sop[:, 1]),
        }
        return {k: jax.device_put(v, devs[c]) for k, v in percore.items()}

    with ThreadPoolExecutor(max_workers=N_CORES) as pool:
        percores = list(pool.map(_prep2, range(N_CORES)))
    shards2 = {k: [pc[k] for pc in percores] for k in percores[0]}

    _mark("l2 host prep+h2d dispatch")
    dev2 = _assemble(shards2, p2.sh)
    dev2["hs"] = hs_dev
    outs2 = p2.run(dev2)
    yt = np.asarray(outs2["yt"]).astype(np.float32)   # [8*128, 8, 1024]
    _mark("l2 run+yt fetch")

    # ---- host un-swizzle + output affine
    # yt[c] is [128(q), 8(blk), 1024]: row of token (128*blk + q)
    y = np.empty((N_CORES * NTOK, 1024), dtype=np.float32)
    for c in range(N_CORES):
        blk = yt[c * 128:(c + 1) * 128]
        y[c * NTOK:(c + 1) * NTOK] = blk.transpose(1, 0, 2).reshape(NTOK, 1024)
    y = y * s_dn + bias_dn[None, :]
    return y.reshape(orig_shape).astype(np.float32)


# revision 52
# speedup vs baseline: 1.0040x; 1.0040x over previous
"""KMoE feed-forward on 8 TRN2 NeuronCores.

Two fixed (input-independent) SPMD Bass programs — one per KMoE layer — with
host-side routing.  Layer-1 per-slot expert outputs (h_slots) stay on-device
between the two launches, and every data-dependent token->slot dispatch runs
on-device via gpsimd.dma_gather, so the only host<->device traffic is the
slot-packed x, per-group weight panels, router logits, small index tables and
the final output.  Device matmuls are bf16 with f32 PSUM accumulation.
"""
import numpy as np

D1 = D2 = 32
F1 = F2 = 64
E = 64
TOP_K = 2
N_CORES = 8
G = 8                      # slots per expert group (one weight panel entry)
S = 2560                   # fixed padded slot count (2048 + 64*(G-1) = 2496 max)
NG = S // G
NTOK = 1024                # tokens per core
RCH = 256                  # router token chunk (layer 1)
GCH = 256                  # slot gather chunk (layer 2)

_bf16 = None


def _bf16dt():
    global _bf16
    if _bf16 is None:
        import ml_dtypes
        _bf16 = np.dtype(ml_dtypes.bfloat16)
    return _bf16


# ---------------------------------------------------------------- host glue

def _route(logits):
    idx = np.argpartition(-logits, TOP_K - 1, axis=1)[:, :TOP_K]
    vals = np.take_along_axis(logits, idx, axis=1)
    order = np.argsort(-vals, axis=1, kind="stable")
    idx = np.take_along_axis(idx, order, axis=1)
    vals = np.take_along_axis(vals, order, axis=1)
    ex = np.exp(vals - vals.max(axis=1, keepdims=True))
    probs = ex / ex.sum(axis=1, keepdims=True)
    return idx.astype(np.int64), probs.astype(np.float32)


def _dispatch(idx, probs):
    """Expert-sorted slot assignment for one core."""
    n = idx.shape[0]
    ex = idx.reshape(-1)
    order = np.argsort(ex, kind="stable")
    counts = np.bincount(ex, minlength=E)
    padded = ((counts + G - 1) // G) * G
    total = int(padded.sum())
    assert total <= S, (total, S)
    off_pad = np.zeros(E, dtype=np.int64)
    off_pad[1:] = np.cumsum(padded)[:-1]
    off_raw = np.zeros(E, dtype=np.int64)
    off_raw[1:] = np.cumsum(counts)[:-1]
    ex_sorted = ex[order]
    pos_in_e = np.arange(2 * n) - off_raw[ex_sorted]
    slot_sorted = off_pad[ex_sorted] + pos_in_e
    tok_of_slot = np.zeros(S, dtype=np.int64)
    gate_of_slot = np.zeros(S, dtype=np.float32)
    tok_of_slot[slot_sorted] = order // 2
    gate_of_slot[slot_sorted] = probs.reshape(-1)[order]
    slot_of_pair = np.zeros((n, 2), dtype=np.int64)
    slot_of_pair[order // 2, order % 2] = slot_sorted
    expert_of_group = np.zeros(NG, dtype=np.int64)
    reps = (padded // G).astype(np.int64)
    eog = np.repeat(np.arange(E), reps)
    expert_of_group[: eog.shape[0]] = eog
    return tok_of_slot, gate_of_slot, expert_of_group, slot_of_pair


def _wrap16(v):
    """[n] int -> [128, n//16] int16 dma_gather index layout."""
    v = np.asarray(v, dtype=np.int16)
    lay = v.reshape(-1, 16).T            # [16, n//16]
    return np.ascontiguousarray(np.tile(lay, (8, 1)))


def _exact_logits(X, idx1c, probs1c, A_up, B_up, s_up, bias_up, W_down):
    """Exact f32 layer-2 router logits for a subset of tokens.

    X [M,32,32]; idx1c/probs1c [M,2] layer-1 routing of those tokens."""
    from scipy.special import erf
    Ag = A_up[idx1c]                     # [M,2,64,32]
    Bg = B_up[idx1c]
    U = np.einsum('mkoi,mij->mkoj', Ag, X, optimize=True)
    Y = np.einsum('mkoj,mkpj->mkop', U, Bg, optimize=True)
    h = ((Y * probs1c[:, :, None, None]).sum(1).reshape(X.shape[0], 4096)
         * s_up + bias_up[None, :])
    hg = 0.5 * h * (1.0 + erf(h / np.sqrt(2.0)))
    return (hg @ W_down.T).astype(np.float32)


_QQ = np.arange(128)
_BB32 = np.arange(32)
# h_slots row swizzle: stored flat[128*b + q] = h[(o1, p1)] with
# o1 = b + 32*(q//64), p1 = q%64  (i.e. flat(o1,p1) = 128*(o1%32)+64*(o1//32)+p1)
_FR = ((_BB32[None, :] + 32 * (_QQ[:, None] // 64)) * 64 + _QQ[:, None] % 64)  # [128,32]
_SIGMA = 32 * (np.arange(64) % 2) + np.arange(64) // 2  # wa1 panel column perm


# ------------------------------------------------------------- bass programs

def _build_l1(nc):
    import concourse.mybir as mybir
    import concourse.tile as tile

    bf = mybir.dt.bfloat16
    f32 = mybir.dt.float32
    i16 = mybir.dt.int16

    xs_d = nc.dram_tensor("xs", (32, S * 32), bf, kind="ExternalInput")
    wbp_d = nc.dram_tensor("wbp", (32, NG * 64), bf, kind="ExternalInput")
    wap_d = nc.dram_tensor("wap", (32, NG * 64), bf, kind="ExternalInput")
    wdp_d = nc.dram_tensor("wdp", (128, 32 * 64), bf, kind="ExternalInput")
    bias_d = nc.dram_tensor("biasT", (128, 32), bf, kind="ExternalInput")
    pa_d = nc.dram_tensor("pa", (128, NTOK // 16), i16, kind="ExternalInput")
    pb_d = nc.dram_tensor("pb", (128, NTOK // 16), i16, kind="ExternalInput")
    hso_d = nc.dram_tensor("hso", (NG, G, 64, 64), bf, kind="ExternalOutput")
    lg_d = nc.dram_tensor("lg", (64, NTOK), f32, kind="ExternalOutput")

    with tile.TileContext(nc) as tc:
        with tc.tile_pool(name="const", bufs=1) as cp, \
             tc.tile_pool(name="dram", bufs=1, space="DRAM") as dp, \
             tc.tile_pool(name="xstr", bufs=2) as xsp, \
             tc.tile_pool(name="vs", bufs=3) as vp, \
             tc.tile_pool(name="hp", bufs=3) as hp, \
             tc.tile_pool(name="ga", bufs=2) as gap, \
             tc.tile_pool(name="gb", bufs=2) as gbp, \
             tc.tile_pool(name="lo", bufs=2) as lop, \
             tc.tile_pool(name="sg", bufs=1) as sgp, \
             tc.tile_pool(name="ps1", bufs=2, space="PSUM") as p1, \
             tc.tile_pool(name="ps2", bufs=2, space="PSUM") as p2, \
             tc.tile_pool(name="psr", bufs=2, space="PSUM") as pr:
            hs = dp.tile([NG, G, 64, 64], bf)          # h_slots (internal DRAM)
            wbp = cp.tile([32, NG * 64], bf)
            nc.sync.dma_start(wbp[:], wbp_d[:])
            wap = cp.tile([32, NG * 64], bf)
            nc.sync.dma_start(wap[:], wap_d[:])
            wdp = cp.tile([128, 32 * 64], bf)
            nc.sync.dma_start(wdp[:], wdp_d[:])
            biasT = cp.tile([128, 32], bf)
            nc.sync.dma_start(biasT[:], bias_d[:])
            pa = cp.tile([128, NTOK // 16], i16)
            nc.sync.dma_start(pa[:], pa_d[:])
            pb = cp.tile([128, NTOK // 16], i16)
            nc.sync.dma_start(pb[:], pb_d[:])

            CHG = 16                       # groups per streamed xs chunk
            for g in range(NG):
                if g % CHG == 0:
                    xs = xsp.tile([32, CHG * G * 32], bf, tag="xsch")
                    nc.sync.dma_start(
                        xs[:], xs_d[:, g * G * 32:(g + CHG) * G * 32])
                ps1 = p1.tile([32, 512], mybir.dt.float32)
                for k in range(G):
                    s = (g % CHG) * G + k
                    nc.tensor.matmul(ps1[:, k * 64:(k + 1) * 64],
                                     xs[:, s * 32:(s + 1) * 32],
                                     wbp[:, g * 64:(g + 1) * 64],
                                     start=True, stop=True)
                vsb = vp.tile([32, 512], bf, tag="vsb")
                nc.vector.tensor_copy(vsb[:], ps1[:])
                ps2 = p2.tile([64, 512], mybir.dt.float32)
                nc.tensor.matmul(ps2[:], wap[:, g * 64:(g + 1) * 64], vsb[:],
                                 start=True, stop=True)
                hsb = hp.tile([64, 512], bf, tag="hsb")
                nc.vector.tensor_copy(hsb[:], ps2[:])
                # h_slots row flat = 64*p + j  (p = psum partition, sigma-mapped)
                nc.sync.dma_start(hs[g].rearrange("k p j -> p k j"), hsb[:])

            hs_rows = hs[:].rearrange("g k p j -> (g k) (p j)")
            for c in range(NTOK // RCH):
                ha = gap.tile([128, 32, RCH], bf, tag="ha")
                nc.gpsimd.dma_gather(ha[:], hs_rows,
                                     pa[:, c * (RCH // 16):(c + 1) * (RCH // 16)],
                                     RCH, RCH, 4096, transpose=True)
                hb = gbp.tile([128, 32, RCH], bf, tag="hb")
                nc.gpsimd.dma_gather(hb[:], hs_rows,
                                     pb[:, c * (RCH // 16):(c + 1) * (RCH // 16)],
                                     RCH, RCH, 4096, transpose=True)
                nc.vector.tensor_add(ha[:], ha[:], hb[:])
                bias_b = biasT[:].unsqueeze(2).broadcast_to((128, 32, RCH))
                nc.vector.tensor_add(ha[:], ha[:], bias_b)
                # gelu(x) ~= x * sigmoid(1.702 x); exact to O(5e-5) at |x|<<1
                sg = sgp.tile([128, 32, RCH], bf, tag="sg")
                nc.scalar.activation(sg[:], ha[:],
                                     mybir.ActivationFunctionType.Sigmoid,
                                     scale=1.702)
                nc.vector.tensor_mul(ha[:], ha[:], sg[:])
                pl = pr.tile([64, RCH], mybir.dt.float32)
                for b in range(32):
                    nc.tensor.matmul(pl[:], wdp[:, b * 64:(b + 1) * 64],
                                     ha[:, b, :], start=(b == 0), stop=(b == 31))
                lsb = lop.tile([64, RCH], mybir.dt.float32, tag="lsb")
                nc.vector.tensor_copy(lsb[:], pl[:])
                nc.sync.dma_start(lg_d[:, c * RCH:(c + 1) * RCH], lsb[:])

            # publish h_slots for launch 2 (device-resident output)
            nc.sync.dma_start(hso_d[:], hs[:])
    return nc


def _build_l2(nc):
    import concourse.mybir as mybir
    import concourse.tile as tile

    bf = mybir.dt.bfloat16
    i16 = mybir.dt.int16

    hs_d = nc.dram_tensor("hs", (NG, G, 64, 64), bf, kind="ExternalInput")
    wbp_d = nc.dram_tensor("wbp", (64, NG * 32), bf, kind="ExternalInput")
    wap_d = nc.dram_tensor("wap", (64, NG * 32), bf, kind="ExternalInput")
    bias_d = nc.dram_tensor("biasT", (128, 32), bf, kind="ExternalInput")
    ga_d = nc.dram_tensor("ga", (128, S // 16), i16, kind="ExternalInput")
    gb_d = nc.dram_tensor("gb", (128, S // 16), i16, kind="ExternalInput")
    g2_d = nc.dram_tensor("g2e", (128, S), bf, kind="ExternalInput")
    qa_d = nc.dram_tensor("qa", (128, NTOK // 16), i16, kind="ExternalInput")
    qb_d = nc.dram_tensor("qb", (128, NTOK // 16), i16, kind="ExternalInput")
    yt_d = nc.dram_tensor("yt", (128, 8, NTOK), bf, kind="ExternalOutput")

    with tile.TileContext(nc) as tc:
        with tc.tile_pool(name="const", bufs=1) as cp, \
             tc.tile_pool(name="dram", bufs=1, space="DRAM") as dp, \
             tc.tile_pool(name="xa", bufs=2) as xap, \
             tc.tile_pool(name="xb", bufs=2) as xbp, \
             tc.tile_pool(name="vs", bufs=3) as vp, \
             tc.tile_pool(name="ys", bufs=3) as yp, \
             tc.tile_pool(name="fin", bufs=1) as fp, \
             tc.tile_pool(name="sg", bufs=1) as sgp, \
             tc.tile_pool(name="ps1", bufs=2, space="PSUM") as p1, \
             tc.tile_pool(name="ps2", bufs=2, space="PSUM") as p2:
            ys = dp.tile([S // 16, 16, 32, 32], bf)    # y_slots (internal DRAM)
            wb2 = cp.tile([128, NG * 32], bf)
            nc.sync.dma_start(wb2[0:64, :], wbp_d[:])
            nc.sync.dma_start(wb2[64:128, :], wbp_d[:])
            wa2 = cp.tile([64, NG * 32], bf)
            nc.sync.dma_start(wa2[:], wap_d[:])
            biasT = cp.tile([128, 32], bf)
            nc.sync.dma_start(biasT[:], bias_d[:])
            g2e = cp.tile([128, S], bf)
            nc.sync.dma_start(g2e[:], g2_d[:])
            ga = cp.tile([128, S // 16], i16)
            nc.sync.dma_start(ga[:], ga_d[:])
            gb = cp.tile([128, S // 16], i16)
            nc.sync.dma_start(gb[:], gb_d[:])
            qa = cp.tile([128, NTOK // 16], i16)
            nc.sync.dma_start(qa[:], qa_d[:])
            qb = cp.tile([128, NTOK // 16], i16)
            nc.sync.dma_start(qb[:], qb_d[:])

            hs_rows = hs_d[:].rearrange("g k p j -> (g k) (p j)")
            for c in range(S // GCH):
                xa = xap.tile([128, 32, GCH], bf, tag="xa")
                nc.gpsimd.dma_gather(xa[:], hs_rows,
                                     ga[:, c * (GCH // 16):(c + 1) * (GCH // 16)],
                                     GCH, GCH, 4096, transpose=True)
                xb = xbp.tile([128, 32, GCH], bf, tag="xb")
                nc.gpsimd.dma_gather(xb[:], hs_rows,
                                     gb[:, c * (GCH // 16):(c + 1) * (GCH // 16)],
                                     GCH, GCH, 4096, transpose=True)
                nc.vector.tensor_add(xa[:], xa[:], xb[:])
                bias_b = biasT[:].unsqueeze(2).broadcast_to((128, 32, GCH))
                nc.vector.tensor_add(xa[:], xa[:], bias_b)
                sg = sgp.tile([128, 32, GCH], bf, tag="sg")
                nc.scalar.activation(sg[:], xa[:],
                                     mybir.ActivationFunctionType.Sigmoid,
                                     scale=1.702)
                nc.vector.tensor_mul(xa[:], xa[:], sg[:])
                gate_b = (g2e[:, c * GCH:(c + 1) * GCH]
                          .unsqueeze(1).broadcast_to((128, 32, GCH)))
                nc.vector.tensor_mul(xa[:], xa[:], gate_b)
                for pair in range(GCH // 16):
                    ps1 = p1.tile([64, 512], mybir.dt.float32)
                    for k in range(16):
                        sl = pair * 16 + k
                        gof = (c * GCH + sl) // G
                        nc.tensor.matmul(ps1[0:32, k * 32:(k + 1) * 32],
                                         xa[0:64, :, sl],
                                         wb2[0:64, gof * 32:(gof + 1) * 32],
                                         start=True, stop=True)
                        nc.tensor.matmul(ps1[32:64, k * 32:(k + 1) * 32],
                                         xa[64:128, :, sl],
                                         wb2[64:128, gof * 32:(gof + 1) * 32],
                                         start=True, stop=True)
                    vsb = vp.tile([64, 512], bf, tag="vsb")
                    nc.vector.tensor_copy(vsb[:], ps1[:])
                    ps2 = p2.tile([32, 512], mybir.dt.float32)
                    for t in range(2):
                        gof = (c * GCH) // G + pair * 2 + t
                        nc.tensor.matmul(ps2[:, t * 256:(t + 1) * 256],
                                         wa2[:, gof * 32:(gof + 1) * 32],
                                         vsb[:, t * 256:(t + 1) * 256],
                                         start=True, stop=True)
                    ysb = yp.tile([32, 512], bf, tag="ysb")
                    nc.vector.tensor_copy(ysb[:], ps2[:])
                    blk = (c * GCH) // 16 + pair
                    nc.sync.dma_start(ys[blk].rearrange("k o p -> o k p"), ysb[:])

            # non-transpose gather: ya[t%128, t//128, :] = y_slots[qa[t], :]
            # (transpose-mode faults for 2048B rows on this runtime)
            ys_rows = ys[:].rearrange("g k o p -> (g k) (o p)")
            ya = fp.tile([128, 8, NTOK], bf, tag="ya")
            nc.gpsimd.dma_gather(ya[:], ys_rows, qa[:], NTOK, NTOK, 1024)
            yb = fp.tile([128, 8, NTOK], bf, tag="yb")
            nc.gpsimd.dma_gather(yb[:], ys_rows, qb[:], NTOK, NTOK, 1024)
            nc.vector.tensor_add(ya[:], ya[:], yb[:])
            nc.sync.dma_start(yt_d[:], ya[:])
    return nc


# ------------------------------------------------------------------ runner

_MESH = None


def _mesh():
    global _MESH
    if _MESH is None:
        import jax
        from jax.sharding import Mesh
        _MESH = Mesh(np.asarray(jax.devices()[:N_CORES]), ("core",))
    return _MESH


class _Prog:
    """A finalized bass program with an AOT-compiled sharded executable."""

    def __init__(self, name, build_fn):
        import jax
        import concourse.bacc as bacc
        import concourse.mybir as mybir
        from concourse import bass2jax
        from jax.sharding import PartitionSpec, NamedSharding
        from jax.experimental.shard_map import shard_map

        bass2jax.install_neuronx_cc_hook()
        nc = bacc.Bacc(name=name, trn_type="TRN2")
        build_fn(nc)
        nc.finalize()
        self.nc = nc
        mesh = _mesh()
        P = PartitionSpec
        self.sh = NamedSharding(mesh, P("core"))

        partition_name = (nc.partition_id_tensor.name
                          if nc.partition_id_tensor else None)
        in_names, out_names, out_avals = [], [], []
        for alloc in nc.m.functions[0].allocations:
            if not isinstance(alloc, mybir.MemoryLocationSet):
                continue
            nm = alloc.memorylocations[0].name
            if alloc.kind == "ExternalInput":
                if nm != partition_name:
                    in_names.append(nm)
            elif alloc.kind == "ExternalOutput":
                out_names.append(nm)
                out_avals.append(jax.core.ShapedArray(
                    tuple(alloc.tensor_shape), mybir.dt.np(alloc.dtype)))
        self.in_names, self.out_names, self.out_avals = \
            in_names, out_names, out_avals
        n_params = len(in_names)
        all_in = in_names + out_names
        if partition_name is not None:
            all_in.append(partition_name)
        all_in_names = tuple(all_in)

        def _body(*args):
            operands = list(args)
            if partition_name is not None:
                operands.append(bass2jax.partition_id_tensor())
            outs = bass2jax._bass_exec_p.bind(
                *operands, out_avals=tuple(out_avals), in_names=all_in_names,
                out_names=tuple(out_names), lowering_input_output_aliases=(),
                sim_require_finite=True, sim_require_nnan=True, nc=nc)
            return tuple(outs)

        donate = tuple(range(n_params, n_params + len(out_names)))
        sharded = jax.jit(
            shard_map(_body, mesh=mesh,
                      in_specs=(P("core"),) * (n_params + len(out_names)),
                      out_specs=(P("core"),) * len(out_names), check_rep=False),
            donate_argnums=donate, keep_unused=True)
        avals = []
        for nm in in_names:
            a = self._gshape(nc, nm)
            avals.append(a)
        for a2 in out_avals:
            avals.append(jax.ShapeDtypeStruct(
                (N_CORES * a2.shape[0],) + tuple(a2.shape[1:]), a2.dtype,
                sharding=self.sh))
        self.compiled = sharded.lower(*avals).compile()

    @staticmethod
    def _gshape(nc, nm):
        import jax
        import concourse.mybir as mybir
        for alloc in nc.m.functions[0].allocations:
            if (isinstance(alloc, mybir.MemoryLocationSet)
                    and alloc.memorylocations[0].name == nm):
                shp = tuple(alloc.tensor_shape)
                return jax.ShapeDtypeStruct(
                    (N_CORES * shp[0],) + shp[1:], mybir.dt.np(alloc.dtype))
        raise KeyError(nm)

    def run(self, dev_ins):
        import jax
        import jax.numpy as jnp
        ins = [dev_ins[nm] for nm in self.in_names]
        zeros = [
            jax.jit(lambda s=tuple(a.shape), d=a.dtype:
                    jnp.zeros((N_CORES * s[0],) + s[1:], d),
                    out_shardings=self.sh)()
            for a in self.out_avals
        ]
        outs = self.compiled(*ins, *zeros)
        return dict(zip(self.out_names, outs))


_PROGS = None


def _progs():
    global _PROGS
    if _PROGS is None:
        _PROGS = (_Prog("kmoe_l1", _build_l1), _Prog("kmoe_l2", _build_l2))
    return _PROGS


try:
    _progs()          # import-time init: device discovery, program build, AOT
except Exception:     # fall back to lazy init inside kernel()
    _PROGS = None


# ------------------------------------------------------------------ kernel

def kernel(x, W_up, A_up, B_up, scale_up, bias_up,
           W_down, A_down, B_down, scale_down, bias_down):
    import sys
    import time
    import concourse.bacc as bacc

    _t = [time.time()]

    def _mark(tag):
        now = time.time()
        print(f"[kmoe] {tag}: {now - _t[0]:.2f}s", file=sys.stderr)
        _t[0] = now

    bf16 = _bf16dt()
    x = np.asarray(x, np.float32)
    orig_shape = x.shape
    x_flat = np.ascontiguousarray(x.reshape(-1, D1 * D2))
    W_up = np.asarray(W_up, np.float32)
    A_up = np.asarray(A_up, np.float32)
    B_up = np.asarray(B_up, np.float32)
    W_down = np.asarray(W_down, np.float32)
    A_down = np.asarray(A_down, np.float32)
    B_down = np.asarray(B_down, np.float32)
    s_up = float(np.asarray(scale_up).reshape(-1)[0])
    s_dn = float(np.asarray(scale_down).reshape(-1)[0])
    bias_up = np.asarray(bias_up, np.float32).reshape(-1)
    bias_dn = np.asarray(bias_down, np.float32).reshape(-1)

    # ---- layer-1 routing + dispatch (host, f32 exact)
    logits1 = x_flat.reshape(-1, D1 * D2) @ W_up.T
    idx1, probs1 = _route(logits1)

    wb1_all = B_up.transpose(0, 2, 1)                         # [E, 32(j), 64(p)]
    wa1_all = (A_up[:, _SIGMA, :] * s_up).transpose(0, 2, 1)  # [E, 32(i), 64(o')]

    # router panel:  wdp[q, b*64+e] = W_down[e, _FR[q,b]]
    wdp = np.ascontiguousarray(
        W_down[:, _FR].transpose(1, 2, 0)).reshape(128, 32 * 64).astype(bf16)
    biasT = np.ascontiguousarray(bias_up[_FR]).astype(bf16)

    import jax
    p1, p2 = _progs()
    devs = list(_mesh().devices.flat)

    def _assemble(shards_by_name, sh):
        out = {}
        for k, shards in shards_by_name.items():
            s0 = shards[0].shape
            out[k] = jax.make_array_from_single_device_arrays(
                (N_CORES * s0[0],) + tuple(s0[1:]), sh, shards)
        return out

    # per-core prep in a thread pool with put-as-you-go: numpy prep for all
    # cores runs concurrently (GIL released in big array ops) and each core's
    # tunnel transfer starts the moment its arrays are ready
    from concurrent.futures import ThreadPoolExecutor

    sop1_all = [None] * N_CORES

    def _prep1(c):
        t0 = c * NTOK
        tok, gate, eog, sop = _dispatch(idx1[t0:t0 + NTOK], probs1[t0:t0 + NTOK])
        sop1_all[c] = sop
        xc = x_flat[t0:t0 + NTOK]
        xst = xc.reshape(NTOK, 32, 32)[tok] * gate[:, None, None]
        xs = np.ascontiguousarray(xst.transpose(2, 0, 1)).reshape(32, S * 32)
        percore = {
            "xs": xs.astype(bf16),
            "wbp": np.ascontiguousarray(
                wb1_all[eog].transpose(1, 0, 2)).reshape(32, NG * 64).astype(bf16),
            "wap": np.ascontiguousarray(
                wa1_all[eog].transpose(1, 0, 2)).reshape(32, NG * 64).astype(bf16),
            "wdp": wdp, "biasT": biasT,
            "pa": _wrap16(sop[:, 0]), "pb": _wrap16(sop[:, 1]),
        }
        return {k: jax.device_put(v, devs[c]) for k, v in percore.items()}

    with ThreadPoolExecutor(max_workers=N_CORES) as pool:
        percores = list(pool.map(_prep1, range(N_CORES)))
    shards1 = {k: [pc[k] for pc in percores] for k in percores[0]}

    _mark("l1 host prep+h2d dispatch")
    dev1 = _assemble(shards1, p1.sh)
    outs1 = p1.run(dev1)
    hs_dev = outs1["hso"]                    # stays on device
    logits2 = np.asarray(outs1["lg"])        # [8*64, 1024] f32
    _mark("l1 run+lg fetch")

    # ---- layer-2 routing (host) from device logits
    logits2 = logits2.reshape(N_CORES, 64, NTOK).transpose(0, 2, 1)

    wb2_all = B_down.transpose(0, 2, 1)      # [E, 64(j), 32(p)]
    wa2_all = A_down.transpose(0, 2, 1)      # [E, 64(i), 32(o)]

    def _prep2(c):
        lgc = np.ascontiguousarray(logits2[c])          # [NTOK, 64]
        # device logits carry bf16-level noise; near-tied top-2/3 tokens get
        # exact f32 logits recomputed on host so expert selection matches f32.
        top3 = np.argpartition(-lgc, 2, axis=1)[:, :3]
        v3 = np.sort(np.take_along_axis(lgc, top3, axis=1), axis=1)
        gap23 = v3[:, 1] - v3[:, 0]                     # 2nd minus 3rd largest
        tau = 0.02 * float(lgc.std())
        amb = np.nonzero(gap23 < tau)[0]
        if amb.size:
            t0c = c * NTOK
            Xa = x_flat[t0c:t0c + NTOK].reshape(NTOK, 32, 32)[amb]
            lgc[amb] = _exact_logits(
                Xa, idx1[t0c + amb], probs1[t0c + amb],
                A_up, B_up, s_up, bias_up, W_down)
        idx2, probs2 = _route(lgc)
        tok, gate, eog, sop = _dispatch(idx2, probs2)
        sop1 = sop1_all[c]
        percore = {
            "wbp": np.ascontiguousarray(
                wb2_all[eog].transpose(1, 0, 2)).reshape(64, NG * 32).astype(bf16),
            "wap": np.ascontiguousarray(
                wa2_all[eog].transpose(1, 0, 2)).reshape(64, NG * 32).astype(bf16),
            "biasT": biasT,
            "ga": _wrap16(sop1[tok, 0]),
            "gb": _wrap16(sop1[tok, 1]),
            "g2e": np.ascontiguousarray(
                np.broadcast_to(gate[None, :], (128, S))).astype(bf16),
            "qa": _wrap16(sop[:, 0]),
            "qb": _wrap16(sop[:, 1]),
        }
        return {k: jax.device_put(v, devs[c]) for k, v in percore.items()}

    with ThreadPoolExecutor(max_workers=N_CORES) as pool:
        percores = list(pool.map(_prep2, range(N_CORES)))
    shards2 = {k: [pc[k] for pc in percores] for k in percores[0]}

    _mark("l2 host prep+h2d dispatch")
    dev2 = _assemble(shards2, p2.sh)
    dev2["hs"] = hs_dev
    outs2 = p2.run(dev2)
    yt = np.asarray(outs2["yt"]).astype(np.float32)   # [8*128, 8, 1024]
    _mark("l2 run+yt fetch")

    # ---- host un-swizzle + output affine
    # yt[c] is [128(q), 8(blk), 1024]: row of token (128*blk + q)
    y = np.empty((N_CORES * NTOK, 1024), dtype=np.float32)
    for c in range(N_CORES):
        blk = yt[c * 128:(c + 1) * 128]
        y[c * NTOK:(c + 1) * NTOK] = blk.transpose(1, 0, 2).reshape(NTOK, 1024)
    y = y * s_dn + bias_dn[None, :]
    return y.reshape(orig_shape).astype(np.float32)


# revision 54
# speedup vs baseline: 1.0233x; 1.0193x over previous
"""KMoE feed-forward on 8 TRN2 NeuronCores.

Two fixed (input-independent) SPMD Bass programs — one per KMoE layer — with
host-side routing.  Layer-1 per-slot expert outputs (h_slots) stay on-device
between the two launches, and every data-dependent token->slot dispatch runs
on-device via gpsimd.dma_gather, so the only host<->device traffic is the
slot-packed x, per-group weight panels, router logits, small index tables and
the final output.  Device matmuls are bf16 with f32 PSUM accumulation.
"""
import numpy as np

D1 = D2 = 32
F1 = F2 = 64
E = 64
TOP_K = 2
N_CORES = 8
G = 8                      # slots per expert group (one weight panel entry)
S = 2560                   # fixed padded slot count (2048 + 64*(G-1) = 2496 max)
NG = S // G
NTOK = 1024                # tokens per core
RCH = 256                  # router token chunk (layer 1)
GCH = 256                  # slot gather chunk (layer 2)

_bf16 = None


def _bf16dt():
    global _bf16
    if _bf16 is None:
        import ml_dtypes
        _bf16 = np.dtype(ml_dtypes.bfloat16)
    return _bf16


# ---------------------------------------------------------------- host glue

def _route(logits):
    idx = np.argpartition(-logits, TOP_K - 1, axis=1)[:, :TOP_K]
    vals = np.take_along_axis(logits, idx, axis=1)
    order = np.argsort(-vals, axis=1, kind="stable")
    idx = np.take_along_axis(idx, order, axis=1)
    vals = np.take_along_axis(vals, order, axis=1)
    ex = np.exp(vals - vals.max(axis=1, keepdims=True))
    probs = ex / ex.sum(axis=1, keepdims=True)
    return idx.astype(np.int64), probs.astype(np.float32)


def _dispatch(idx, probs):
    """Expert-sorted slot assignment for one core."""
    n = idx.shape[0]
    ex = idx.reshape(-1)
    order = np.argsort(ex, kind="stable")
    counts = np.bincount(ex, minlength=E)
    padded = ((counts + G - 1) // G) * G
    total = int(padded.sum())
    assert total <= S, (total, S)
    off_pad = np.zeros(E, dtype=np.int64)
    off_pad[1:] = np.cumsum(padded)[:-1]
    off_raw = np.zeros(E, dtype=np.int64)
    off_raw[1:] = np.cumsum(counts)[:-1]
    ex_sorted = ex[order]
    pos_in_e = np.arange(2 * n) - off_raw[ex_sorted]
    slot_sorted = off_pad[ex_sorted] + pos_in_e
    tok_of_slot = np.zeros(S, dtype=np.int64)
    gate_of_slot = np.zeros(S, dtype=np.float32)
    tok_of_slot[slot_sorted] = order // 2
    gate_of_slot[slot_sorted] = probs.reshape(-1)[order]
    slot_of_pair = np.zeros((n, 2), dtype=np.int64)
    slot_of_pair[order // 2, order % 2] = slot_sorted
    expert_of_group = np.zeros(NG, dtype=np.int64)
    reps = (padded // G).astype(np.int64)
    eog = np.repeat(np.arange(E), reps)
    expert_of_group[: eog.shape[0]] = eog
    return tok_of_slot, gate_of_slot, expert_of_group, slot_of_pair


def _wrap16(v):
    """[n] int -> [128, n//16] int16 dma_gather index layout."""
    v = np.asarray(v, dtype=np.int16)
    lay = v.reshape(-1, 16).T            # [16, n//16]
    return np.ascontiguousarray(np.tile(lay, (8, 1)))


def _exact_logits(X, idx1c, probs1c, A_up, B_up, s_up, bias_up, W_down):
    """Exact f32 layer-2 router logits for a subset of tokens.

    X [M,32,32]; idx1c/probs1c [M,2] layer-1 routing of those tokens."""
    from scipy.special import erf
    Ag = A_up[idx1c]                     # [M,2,64,32]
    Bg = B_up[idx1c]
    U = np.einsum('mkoi,mij->mkoj', Ag, X, optimize=True)
    Y = np.einsum('mkoj,mkpj->mkop', U, Bg, optimize=True)
    h = ((Y * probs1c[:, :, None, None]).sum(1).reshape(X.shape[0], 4096)
         * s_up + bias_up[None, :])
    hg = 0.5 * h * (1.0 + erf(h / np.sqrt(2.0)))
    return (hg @ W_down.T).astype(np.float32)


_QQ = np.arange(128)
_BB32 = np.arange(32)
# h_slots row swizzle: stored flat[128*b + q] = h[(o1, p1)] with
# o1 = b + 32*(q//64), p1 = q%64  (i.e. flat(o1,p1) = 128*(o1%32)+64*(o1//32)+p1)
_FR = ((_BB32[None, :] + 32 * (_QQ[:, None] // 64)) * 64 + _QQ[:, None] % 64)  # [128,32]
_SIGMA = 32 * (np.arange(64) % 2) + np.arange(64) // 2  # wa1 panel column perm


# ------------------------------------------------------------- bass programs

def _build_l1(nc):
    import concourse.mybir as mybir
    import concourse.tile as tile

    bf = mybir.dt.bfloat16
    f32 = mybir.dt.float32
    i16 = mybir.dt.int16

    xs_d = nc.dram_tensor("xs", (32, S * 32), bf, kind="ExternalInput")
    wbp_d = nc.dram_tensor("wbp", (32, NG * 64), bf, kind="ExternalInput")
    wap_d = nc.dram_tensor("wap", (32, NG * 64), bf, kind="ExternalInput")
    wdp_d = nc.dram_tensor("wdp", (128, 32 * 64), bf, kind="ExternalInput")
    bias_d = nc.dram_tensor("biasT", (128, 32), bf, kind="ExternalInput")
    pa_d = nc.dram_tensor("pa", (128, NTOK // 16), i16, kind="ExternalInput")
    pb_d = nc.dram_tensor("pb", (128, NTOK // 16), i16, kind="ExternalInput")
    hso_d = nc.dram_tensor("hso", (NG, G, 64, 64), bf, kind="ExternalOutput")
    lg_d = nc.dram_tensor("lg", (64, NTOK), f32, kind="ExternalOutput")

    with tile.TileContext(nc) as tc:
        with tc.tile_pool(name="const", bufs=1) as cp, \
             tc.tile_pool(name="dram", bufs=1, space="DRAM") as dp, \
             tc.tile_pool(name="xstr", bufs=2) as xsp, \
             tc.tile_pool(name="vs", bufs=3) as vp, \
             tc.tile_pool(name="hp", bufs=3) as hp, \
             tc.tile_pool(name="ga", bufs=2) as gap, \
             tc.tile_pool(name="gb", bufs=2) as gbp, \
             tc.tile_pool(name="lo", bufs=2) as lop, \
             tc.tile_pool(name="sg", bufs=1) as sgp, \
             tc.tile_pool(name="ps1", bufs=2, space="PSUM") as p1, \
             tc.tile_pool(name="ps2", bufs=2, space="PSUM") as p2, \
             tc.tile_pool(name="psr", bufs=2, space="PSUM") as pr:
            hs = dp.tile([NG, G, 64, 64], bf)          # h_slots (internal DRAM)
            wbp = cp.tile([32, NG * 64], bf)
            nc.sync.dma_start(wbp[:], wbp_d[:])
            wap = cp.tile([32, NG * 64], bf)
            nc.sync.dma_start(wap[:], wap_d[:])
            wdp = cp.tile([128, 32 * 64], bf)
            nc.sync.dma_start(wdp[:], wdp_d[:])
            biasT = cp.tile([128, 32], bf)
            nc.sync.dma_start(biasT[:], bias_d[:])
            pa = cp.tile([128, NTOK // 16], i16)
            nc.sync.dma_start(pa[:], pa_d[:])
            pb = cp.tile([128, NTOK // 16], i16)
            nc.sync.dma_start(pb[:], pb_d[:])

            CHG = 16                       # groups per streamed xs chunk
            for g in range(NG):
                if g % CHG == 0:
                    xs = xsp.tile([32, CHG * G * 32], bf, tag="xsch")
                    nc.sync.dma_start(
                        xs[:], xs_d[:, g * G * 32:(g + CHG) * G * 32])
                ps1 = p1.tile([32, 512], mybir.dt.float32)
                for k in range(G):
                    s = (g % CHG) * G + k
                    nc.tensor.matmul(ps1[:, k * 64:(k + 1) * 64],
                                     xs[:, s * 32:(s + 1) * 32],
                                     wbp[:, g * 64:(g + 1) * 64],
                                     start=True, stop=True)
                vsb = vp.tile([32, 512], bf, tag="vsb")
                nc.vector.tensor_copy(vsb[:], ps1[:])
                ps2 = p2.tile([64, 512], mybir.dt.float32)
                nc.tensor.matmul(ps2[:], wap[:, g * 64:(g + 1) * 64], vsb[:],
                                 start=True, stop=True)
                hsb = hp.tile([64, 512], bf, tag="hsb")
                nc.vector.tensor_copy(hsb[:], ps2[:])
                # h_slots row flat = 64*p + j  (p = psum partition, sigma-mapped)
                nc.sync.dma_start(hs[g].rearrange("k p j -> p k j"), hsb[:])

            hs_rows = hs[:].rearrange("g k p j -> (g k) (p j)")
            for c in range(NTOK // RCH):
                ha = gap.tile([128, 32, RCH], bf, tag="ha")
                nc.gpsimd.dma_gather(ha[:], hs_rows,
                                     pa[:, c * (RCH // 16):(c + 1) * (RCH // 16)],
                                     RCH, RCH, 4096, transpose=True)
                hb = gbp.tile([128, 32, RCH], bf, tag="hb")
                nc.gpsimd.dma_gather(hb[:], hs_rows,
                                     pb[:, c * (RCH // 16):(c + 1) * (RCH // 16)],
                                     RCH, RCH, 4096, transpose=True)
                nc.vector.tensor_add(ha[:], ha[:], hb[:])
                bias_b = biasT[:].unsqueeze(2).broadcast_to((128, 32, RCH))
                nc.vector.tensor_add(ha[:], ha[:], bias_b)
                # gelu(x) ~= x * sigmoid(1.702 x); exact to O(5e-5) at |x|<<1
                sg = sgp.tile([128, 32, RCH], bf, tag="sg")
                nc.scalar.activation(sg[:], ha[:],
                                     mybir.ActivationFunctionType.Sigmoid,
                                     scale=1.702)
                nc.vector.tensor_mul(ha[:], ha[:], sg[:])
                pl = pr.tile([64, RCH], mybir.dt.float32)
                for b in range(32):
                    nc.tensor.matmul(pl[:], wdp[:, b * 64:(b + 1) * 64],
                                     ha[:, b, :], start=(b == 0), stop=(b == 31))
                lsb = lop.tile([64, RCH], mybir.dt.float32, tag="lsb")
                nc.vector.tensor_copy(lsb[:], pl[:])
                nc.sync.dma_start(lg_d[:, c * RCH:(c + 1) * RCH], lsb[:])

            # publish h_slots for launch 2 (device-resident output)
            nc.sync.dma_start(hso_d[:], hs[:])
    return nc


def _build_l2(nc):
    import concourse.mybir as mybir
    import concourse.tile as tile

    bf = mybir.dt.bfloat16
    i16 = mybir.dt.int16

    hs_d = nc.dram_tensor("hs", (NG, G, 64, 64), bf, kind="ExternalInput")
    wbp_d = nc.dram_tensor("wbp", (64, NG * 32), bf, kind="ExternalInput")
    wap_d = nc.dram_tensor("wap", (64, NG * 32), bf, kind="ExternalInput")
    bias_d = nc.dram_tensor("biasT", (128, 32), bf, kind="ExternalInput")
    ga_d = nc.dram_tensor("ga", (128, S // 16), i16, kind="ExternalInput")
    gb_d = nc.dram_tensor("gb", (128, S // 16), i16, kind="ExternalInput")
    g2_d = nc.dram_tensor("g2e", (128, S), bf, kind="ExternalInput")
    qa_d = nc.dram_tensor("qa", (128, NTOK // 16), i16, kind="ExternalInput")
    qb_d = nc.dram_tensor("qb", (128, NTOK // 16), i16, kind="ExternalInput")
    yt_d = nc.dram_tensor("yt", (128, 8, NTOK), bf, kind="ExternalOutput")

    with tile.TileContext(nc) as tc:
        with tc.tile_pool(name="const", bufs=1) as cp, \
             tc.tile_pool(name="dram", bufs=1, space="DRAM") as dp, \
             tc.tile_pool(name="xa", bufs=2) as xap, \
             tc.tile_pool(name="xb", bufs=2) as xbp, \
             tc.tile_pool(name="vs", bufs=3) as vp, \
             tc.tile_pool(name="ys", bufs=3) as yp, \
             tc.tile_pool(name="fin", bufs=1) as fp, \
             tc.tile_pool(name="sg", bufs=1) as sgp, \
             tc.tile_pool(name="ps1", bufs=2, space="PSUM") as p1, \
             tc.tile_pool(name="ps2", bufs=2, space="PSUM") as p2:
            ys = dp.tile([S // 16, 16, 32, 32], bf)    # y_slots (internal DRAM)
            wb2 = cp.tile([128, NG * 32], bf)
            nc.sync.dma_start(wb2[0:64, :], wbp_d[:])
            nc.sync.dma_start(wb2[64:128, :], wbp_d[:])
            wa2 = cp.tile([64, NG * 32], bf)
            nc.sync.dma_start(wa2[:], wap_d[:])
            biasT = cp.tile([128, 32], bf)
            nc.sync.dma_start(biasT[:], bias_d[:])
            g2e = cp.tile([128, S], bf)
            nc.sync.dma_start(g2e[:], g2_d[:])
            ga = cp.tile([128, S // 16], i16)
            nc.sync.dma_start(ga[:], ga_d[:])
            gb = cp.tile([128, S // 16], i16)
            nc.sync.dma_start(gb[:], gb_d[:])
            qa = cp.tile([128, NTOK // 16], i16)
            nc.sync.dma_start(qa[:], qa_d[:])
            qb = cp.tile([128, NTOK // 16], i16)
            nc.sync.dma_start(qb[:], qb_d[:])

            hs_rows = hs_d[:].rearrange("g k p j -> (g k) (p j)")
            for c in range(S // GCH):
                xa = xap.tile([128, 32, GCH], bf, tag="xa")
                nc.gpsimd.dma_gather(xa[:], hs_rows,
                                     ga[:, c * (GCH // 16):(c + 1) * (GCH // 16)],
                                     GCH, GCH, 4096, transpose=True)
                xb = xbp.tile([128, 32, GCH], bf, tag="xb")
                nc.gpsimd.dma_gather(xb[:], hs_rows,
                                     gb[:, c * (GCH // 16):(c + 1) * (GCH // 16)],
                                     GCH, GCH, 4096, transpose=True)
                nc.vector.tensor_add(xa[:], xa[:], xb[:])
                bias_b = biasT[:].unsqueeze(2).broadcast_to((128, 32, GCH))
                nc.vector.tensor_add(xa[:], xa[:], bias_b)
                sg = sgp.tile([128, 32, GCH], bf, tag="sg")
                nc.scalar.activation(sg[:], xa[:],
                                     mybir.ActivationFunctionType.Sigmoid,
                                     scale=1.702)
                nc.vector.tensor_mul(xa[:], xa[:], sg[:])
                gate_b = (g2e[:, c * GCH:(c + 1) * GCH]
                          .unsqueeze(1).broadcast_to((128, 32, GCH)))
                nc.vector.tensor_mul(xa[:], xa[:], gate_b)
                for pair in range(GCH // 16):
                    ps1 = p1.tile([64, 512], mybir.dt.float32)
                    for k in range(16):
                        sl = pair * 16 + k
                        gof = (c * GCH + sl) // G
                        nc.tensor.matmul(ps1[0:32, k * 32:(k + 1) * 32],
                                         xa[0:64, :, sl],
                                         wb2[0:64, gof * 32:(gof + 1) * 32],
                                         start=True, stop=True)
                        nc.tensor.matmul(ps1[32:64, k * 32:(k + 1) * 32],
                                         xa[64:128, :, sl],
                                         wb2[64:128, gof * 32:(gof + 1) * 32],
                                         start=True, stop=True)
                    vsb = vp.tile([64, 512], bf, tag="vsb")
                    nc.vector.tensor_copy(vsb[:], ps1[:])
                    ps2 = p2.tile([32, 512], mybir.dt.float32)
                    for t in range(2):
                        gof = (c * GCH) // G + pair * 2 + t
                        nc.tensor.matmul(ps2[:, t * 256:(t + 1) * 256],
                                         wa2[:, gof * 32:(gof + 1) * 32],
                                         vsb[:, t * 256:(t + 1) * 256],
                                         start=True, stop=True)
                    ysb = yp.tile([32, 512], bf, tag="ysb")
                    nc.vector.tensor_copy(ysb[:], ps2[:])
                    blk = (c * GCH) // 16 + pair
                    nc.sync.dma_start(ys[blk].rearrange("k o p -> o k p"), ysb[:])

            # non-transpose gather: ya[t%128, t//128, :] = y_slots[qa[t], :]
            # (transpose-mode faults for 2048B rows on this runtime)
            ys_rows = ys[:].rearrange("g k o p -> (g k) (o p)")
            ya = fp.tile([128, 8, NTOK], bf, tag="ya")
            nc.gpsimd.dma_gather(ya[:], ys_rows, qa[:], NTOK, NTOK, 1024)
            yb = fp.tile([128, 8, NTOK], bf, tag="yb")
            nc.gpsimd.dma_gather(yb[:], ys_rows, qb[:], NTOK, NTOK, 1024)
            nc.vector.tensor_add(ya[:], ya[:], yb[:])
            nc.sync.dma_start(yt_d[:], ya[:])
    return nc


# ------------------------------------------------------------------ runner

_MESH = None


def _mesh():
    global _MESH
    if _MESH is None:
        import jax
        from jax.sharding import Mesh
        _MESH = Mesh(np.asarray(jax.devices()[:N_CORES]), ("core",))
    return _MESH


class _Prog:
    """A finalized bass program with an AOT-compiled sharded executable."""

    def __init__(self, name, build_fn):
        import jax
        import concourse.bacc as bacc
        import concourse.mybir as mybir
        from concourse import bass2jax
        from jax.sharding import PartitionSpec, NamedSharding
        from jax.experimental.shard_map import shard_map

        bass2jax.install_neuronx_cc_hook()
        nc = bacc.Bacc(name=name, trn_type="TRN2")
        build_fn(nc)
        nc.finalize()
        self.nc = nc
        mesh = _mesh()
        P = PartitionSpec
        self.sh = NamedSharding(mesh, P("core"))

        partition_name = (nc.partition_id_tensor.name
                          if nc.partition_id_tensor else None)
        in_names, out_names, out_avals = [], [], []
        for alloc in nc.m.functions[0].allocations:
            if not isinstance(alloc, mybir.MemoryLocationSet):
                continue
            nm = alloc.memorylocations[0].name
            if alloc.kind == "ExternalInput":
                if nm != partition_name:
                    in_names.append(nm)
            elif alloc.kind == "ExternalOutput":
                out_names.append(nm)
                out_avals.append(jax.core.ShapedArray(
                    tuple(alloc.tensor_shape), mybir.dt.np(alloc.dtype)))
        self.in_names, self.out_names, self.out_avals = \
            in_names, out_names, out_avals
        n_params = len(in_names)
        all_in = in_names + out_names
        if partition_name is not None:
            all_in.append(partition_name)
        all_in_names = tuple(all_in)

        def _body(*args):
            operands = list(args)
            if partition_name is not None:
                operands.append(bass2jax.partition_id_tensor())
            outs = bass2jax._bass_exec_p.bind(
                *operands, out_avals=tuple(out_avals), in_names=all_in_names,
                out_names=tuple(out_names), lowering_input_output_aliases=(),
                sim_require_finite=True, sim_require_nnan=True, nc=nc)
            return tuple(outs)

        donate = tuple(range(n_params, n_params + len(out_names)))
        sharded = jax.jit(
            shard_map(_body, mesh=mesh,
                      in_specs=(P("core"),) * (n_params + len(out_names)),
                      out_specs=(P("core"),) * len(out_names), check_rep=False),
            donate_argnums=donate, keep_unused=True)
        avals = []
        for nm in in_names:
            a = self._gshape(nc, nm)
            avals.append(a)
        for a2 in out_avals:
            avals.append(jax.ShapeDtypeStruct(
                (N_CORES * a2.shape[0],) + tuple(a2.shape[1:]), a2.dtype,
                sharding=self.sh))
        self.compiled = sharded.lower(*avals).compile()

    @staticmethod
    def _gshape(nc, nm):
        import jax
        import concourse.mybir as mybir
        for alloc in nc.m.functions[0].allocations:
            if (isinstance(alloc, mybir.MemoryLocationSet)
                    and alloc.memorylocations[0].name == nm):
                shp = tuple(alloc.tensor_shape)
                return jax.ShapeDtypeStruct(
                    (N_CORES * shp[0],) + shp[1:], mybir.dt.np(alloc.dtype))
        raise KeyError(nm)

    def run(self, dev_ins):
        import jax
        import jax.numpy as jnp
        ins = [dev_ins[nm] for nm in self.in_names]
        zeros = [
            jax.jit(lambda s=tuple(a.shape), d=a.dtype:
                    jnp.zeros((N_CORES * s[0],) + s[1:], d),
                    out_shardings=self.sh)()
            for a in self.out_avals
        ]
        outs = self.compiled(*ins, *zeros)
        return dict(zip(self.out_names, outs))


_PROGS = None


def _progs():
    global _PROGS
    if _PROGS is None:
        _PROGS = (_Prog("kmoe_l1", _build_l1), _Prog("kmoe_l2", _build_l2))
    return _PROGS


try:
    _progs()          # import-time init: device discovery, program build, AOT
except Exception:     # fall back to lazy init inside kernel()
    _PROGS = None


# ------------------------------------------------------------------ kernel

def kernel(x, W_up, A_up, B_up, scale_up, bias_up,
           W_down, A_down, B_down, scale_down, bias_down):
    import sys
    import time
    import concourse.bacc as bacc

    _t = [time.time()]

    def _mark(tag):
        now = time.time()
        print(f"[kmoe] {tag}: {now - _t[0]:.2f}s", file=sys.stderr)
        _t[0] = now

    bf16 = _bf16dt()
    x = np.asarray(x, np.float32)
    orig_shape = x.shape
    x_flat = np.ascontiguousarray(x.reshape(-1, D1 * D2))
    W_up = np.asarray(W_up, np.float32)
    A_up = np.asarray(A_up, np.float32)
    B_up = np.asarray(B_up, np.float32)
    W_down = np.asarray(W_down, np.float32)
    A_down = np.asarray(A_down, np.float32)
    B_down = np.asarray(B_down, np.float32)
    s_up = float(np.asarray(scale_up).reshape(-1)[0])
    s_dn = float(np.asarray(scale_down).reshape(-1)[0])
    bias_up = np.asarray(bias_up, np.float32).reshape(-1)
    bias_dn = np.asarray(bias_down, np.float32).reshape(-1)

    # ---- layer-1 routing + dispatch (host, f32 exact)
    logits1 = x_flat.reshape(-1, D1 * D2) @ W_up.T
    idx1, probs1 = _route(logits1)

    wb1_all = B_up.transpose(0, 2, 1)                         # [E, 32(j), 64(p)]
    wa1_all = (A_up[:, _SIGMA, :] * s_up).transpose(0, 2, 1)  # [E, 32(i), 64(o')]

    # router panel:  wdp[q, b*64+e] = W_down[e, _FR[q,b]]
    wdp = np.ascontiguousarray(
        W_down[:, _FR].transpose(1, 2, 0)).reshape(128, 32 * 64).astype(bf16)
    biasT = np.ascontiguousarray(bias_up[_FR]).astype(bf16)

    import jax
    p1, p2 = _progs()
    devs = list(_mesh().devices.flat)

    def _assemble(shards_by_name, sh):
        out = {}
        for k, shards in shards_by_name.items():
            s0 = shards[0].shape
            out[k] = jax.make_array_from_single_device_arrays(
                (N_CORES * s0[0],) + tuple(s0[1:]), sh, shards)
        return out

    # per-core prep in a thread pool with put-as-you-go: numpy prep for all
    # cores runs concurrently (GIL released in big array ops) and each core's
    # tunnel transfer starts the moment its arrays are ready
    from concurrent.futures import ThreadPoolExecutor

    sop1_all = [None] * N_CORES

    def _prep1(c):
        t0 = c * NTOK
        tok, gate, eog, sop = _dispatch(idx1[t0:t0 + NTOK], probs1[t0:t0 + NTOK])
        sop1_all[c] = sop
        xc = x_flat[t0:t0 + NTOK]
        xst = xc.reshape(NTOK, 32, 32)[tok] * gate[:, None, None]
        xs = np.ascontiguousarray(xst.transpose(2, 0, 1)).reshape(32, S * 32)
        percore = {
            "xs": xs.astype(bf16),
            "wbp": np.ascontiguousarray(
                wb1_all[eog].transpose(1, 0, 2)).reshape(32, NG * 64).astype(bf16),
            "wap": np.ascontiguousarray(
                wa1_all[eog].transpose(1, 0, 2)).reshape(32, NG * 64).astype(bf16),
            "wdp": wdp, "biasT": biasT,
            "pa": _wrap16(sop[:, 0]), "pb": _wrap16(sop[:, 1]),
        }
        return {k: jax.device_put(v, devs[c]) for k, v in percore.items()}

    with ThreadPoolExecutor(max_workers=N_CORES) as pool:
        percores = list(pool.map(_prep1, range(N_CORES)))
    shards1 = {k: [pc[k] for pc in percores] for k in percores[0]}

    _mark("l1 host prep+h2d dispatch")
    dev1 = _assemble(shards1, p1.sh)
    outs1 = p1.run(dev1)
    hs_dev = outs1["hso"]                    # stays on device
    logits2 = np.asarray(outs1["lg"])        # [8*64, 1024] f32
    _mark("l1 run+lg fetch")

    # ---- layer-2 routing (host) from device logits
    logits2 = logits2.reshape(N_CORES, 64, NTOK).transpose(0, 2, 1)

    wb2_all = B_down.transpose(0, 2, 1)      # [E, 64(j), 32(p)]
    wa2_all = A_down.transpose(0, 2, 1)      # [E, 64(i), 32(o)]

    def _prep2(c):
        lgc = np.ascontiguousarray(logits2[c])          # [NTOK, 64]
        # device logits carry bf16-level noise; near-tied top-2/3 tokens get
        # exact f32 logits recomputed on host so expert selection matches f32.
        top3 = np.argpartition(-lgc, 2, axis=1)[:, :3]
        v3 = np.sort(np.take_along_axis(lgc, top3, axis=1), axis=1)
        gap23 = v3[:, 1] - v3[:, 0]                     # 2nd minus 3rd largest
        tau = 0.02 * float(lgc.std())
        amb = np.nonzero(gap23 < tau)[0]
        if amb.size:
            t0c = c * NTOK
            Xa = x_flat[t0c:t0c + NTOK].reshape(NTOK, 32, 32)[amb]
            lgc[amb] = _exact_logits(
                Xa, idx1[t0c + amb], probs1[t0c + amb],
                A_up, B_up, s_up, bias_up, W_down)
        idx2, probs2 = _route(lgc)
        tok, gate, eog, sop = _dispatch(idx2, probs2)
        sop1 = sop1_all[c]
        percore = {
            "wbp": np.ascontiguousarray(
                wb2_all[eog].transpose(1, 0, 2)).reshape(64, NG * 32).astype(bf16),
            "wap": np.ascontiguousarray(
                wa2_all[eog].transpose(1, 0, 2)).reshape(64, NG * 32).astype(bf16),
            "biasT": biasT,
            "ga": _wrap16(sop1[tok, 0]),
            "gb": _wrap16(sop1[tok, 1]),
            "g2e": np.ascontiguousarray(
                np.broadcast_to(gate[None, :], (128, S))).astype(bf16),
            "qa": _wrap16(sop[:, 0]),
            "qb": _wrap16(sop[:, 1]),
        }
        return {k: jax.device_put(v, devs[c]) for k, v in percore.items()}

    with ThreadPoolExecutor(max_workers=N_CORES) as pool:
        percores = list(pool.map(_prep2, range(N_CORES)))
    shards2 = {k: [pc[k] for pc in percores] for k in percores[0]}

    _mark("l2 host prep+h2d dispatch")
    dev2 = _assemble(shards2, p2.sh)
    dev2["hs"] = hs_dev
    outs2 = p2.run(dev2)
    yt = np.asarray(outs2["yt"]).astype(np.float32)   # [8*128, 8, 1024]
    _mark("l2 run+yt fetch")

    # ---- host un-swizzle + output affine
    # yt[c] is [128(q), 8(blk), 1024]: row of token (128*blk + q)
    y = np.empty((N_CORES * NTOK, 1024), dtype=np.float32)
    for c in range(N_CORES):
        blk = yt[c * 128:(c + 1) * 128]
        y[c * NTOK:(c + 1) * NTOK] = blk.transpose(1, 0, 2).reshape(NTOK, 1024)
    y = y * s_dn + bias_dn[None, :]
    return y.reshape(orig_shape).astype(np.float32)
